# revision 1
# baseline (speedup 1.0000x reference)
"""MinimalGRU (2-layer) Trainium2 Bass kernel, data-parallel over batch on 8 cores.

Full inputs in, full output out. Per core: 4 sequences.
Recurrence keeps h in a group-scattered SBUF layout (partition 32*j+b holds
h[b, 256*j : 256*(j+1)]); gates are computed with h^T as the tiny stationary
operand and the column-permuted W_hh^T as the moving operand across 4 PE
column-groups; h^T for the next step is rebuilt with identity matmuls on 4
concurrent PE row-groups. The two layers run interleaved (layer 1 lags layer
0 by LAG steps) so each layer's elementwise phase hides under the other
layer's matmul phase. Layer-0 outputs never touch DRAM: their transposes
feed the layer-1 input GEMM every 32 steps via a DRAM pre-activation buffer.
"""

import os
import contextlib
import numpy as np
import ml_dtypes

import concourse.bass as bass  # noqa: F401
import concourse.mybir as mybir
from concourse import bacc
from concourse.tile import TileContext
from concourse.bass_utils import run_bass_kernel_spmd

BF16 = ml_dtypes.bfloat16
F32 = np.float32

H = 1024
DX = 512
G = 2 * H          # 2048 gate columns
B = 32
NCORES = 8
BL = B // NCORES   # 4 sequences per core
T = int(os.environ.get("GRU_T", "512"))

PRE_WIN = 4        # pre-activation DMA window (steps)
P1_WIN = 32        # tokens gathered per layer-1 pre GEMM (= steps)
OUT_WIN = 8        # L1 output DMA window (steps)
LAG = 36           # layer-1 step lag behind layer 0 (min safe: 33)

_CACHE: dict = {}


def _perm() -> np.ndarray:
    """Gate-column permutation: group j gets u-cols [256j,256j+256) then
    r-cols [1024+256j, 1024+256j+256)."""
    p = np.empty(G, np.int64)
    for j in range(4):
        p[512 * j: 512 * j + 256] = np.arange(256 * j, 256 * j + 256)
        p[512 * j + 256: 512 * j + 512] = np.arange(H + 256 * j, H + 256 * j + 256)
    return p


class _LS:
    pass


def _make_layer(nc, tc, stack, layer, w_t, pre_d, h0t_t, h0s_t, idt_t,
                wih1p_t=None, b1f_t=None, pre1_d=None, out_dram=None):
    fp32 = mybir.dt.float32
    L = _LS()
    L.layer = layer
    L.w_t = w_t
    L.pre_d = pre_d
    L.idt_t = idt_t
    L.wih1p_t = wih1p_t
    L.b1f_t = b1f_t
    L.pre1_d = pre1_d
    L.out_dram = out_dram
    ctx = stack.enter_context
    L.prew_pool = ctx(tc.tile_pool(name=f"prew{layer}", bufs=2))
    L.stage_pool = ctx(tc.tile_pool(name=f"stage{layer}", bufs=3))
    L.tmp_pool = ctx(tc.tile_pool(name=f"tmp{layer}", bufs=4))
    L.ht_pool = ctx(tc.tile_pool(name=f"ht{layer}", bufs=4))
    L.gps_pool = ctx(tc.tile_pool(name=f"gps{layer}", bufs=2, space="PSUM"))
    L.tps_pool = ctx(tc.tile_pool(name=f"tps{layer}", bufs=1, space="PSUM"))
    if layer == 0:
        L.o0t_pool = ctx(tc.tile_pool(name="o0t0", bufs=2))
        L.pps_pool = ctx(tc.tile_pool(name="pps0", bufs=1, space="PSUM"))
        L.psb_pool = ctx(tc.tile_pool(name="psb0", bufs=2))
    L.gp_tiles = [L.gps_pool.tile([128, 512], fp32, tag="gp",
                                  name=f"gpf{layer}_{i}") for i in range(2)]
    L.prew_tiles = [
        L.prew_pool.tile([128, PRE_WIN, 512], fp32, tag="prew",
                         name=f"prewf{layer}_{i}") for i in range(2)]
    for t_ in L.gp_tiles + L.prew_tiles:
        nc.vector.memset(t_[:], 0.0)
    L.prev_h = h0s_t[:, :]
    L.ht_src = h0t_t
    L.prew_cur = None
    L.stage_cur = None
    L.o0t_cur = None
    return L


def _emit_step(nc, tc, L, ts):
    fp32 = mybir.dt.float32
    bf16 = mybir.dt.bfloat16
    add = mybir.AluOpType.add
    layer = L.layer

    s_pre = ts % PRE_WIN
    if s_pre == 0:
        L.prew_cur = L.prew_tiles[(ts // PRE_WIN) % 2]
        for j in range(4):
            if layer == 0:
                # pre_d: [BL, T, G]; group j wants cols 512j..512j+512
                src = L.pre_d[:, ts: ts + PRE_WIN, 512 * j: 512 * j + 512]
            else:
                # pre_d: [T//P1_WIN, 128, G]; row 4s+b within window
                w = ts // P1_WIN
                s0 = ts % P1_WIN
                src = (L.pre_d[w].rearrange("(s b) g -> b s g", b=4)
                       [:, s0: s0 + PRE_WIN, 512 * j: 512 * j + 512])
            nc.sync.dma_start(L.prew_cur[32 * j: 32 * j + 4, :, :], src)

    if layer == 1:
        s_st = ts % OUT_WIN
        if s_st == 0:
            L.stage_cur = L.stage_pool.tile([128, OUT_WIN, 256], fp32,
                                            tag="stage", name="stagew")
        hn = L.stage_cur[:, s_st, :]
    else:
        L.stage_cur = L.stage_pool.tile([128, 256], fp32, tag="stage",
                                        name="stage")
        hn = L.stage_cur[:, :]
        sw = ts % P1_WIN
        if sw == 0:
            L.o0t_cur = L.o0t_pool.tile([128, 8, 4 * P1_WIN], bf16,
                                        tag="o0t", name="o0t")

    # ---- gate matmuls: gp[32j+b, q] += h[b, kchunk] @ Wp[kchunk, 512j+q]
    gp = L.gp_tiles[ts % 2]
    for k in range(8):
        for j in range(4):
            nc.tensor.matmul(
                gp[32 * j: 32 * j + 4, :],
                (L.ht_src[:, k, :] if L.ht_src.ndim == 3
                 else L.ht_src[:, 4 * k: 4 * k + 4]),
                L.w_t[k][:, 512 * j: 512 * j + 512],
                start=(k == 0), stop=(k == 7),
                tile_position=(0, 32 * j),
                skip_group_check=True,
            )

    # ---- combine: u' = sigmoid(-(gp_u+pre_u));
    #               d = relu(gp_r+pre_r) - h;  hn = h + u'*d
    nc.vector.tensor_tensor(gp[0:100, 0:256], gp[0:100, 0:256],
                            L.prew_cur[0:100, s_pre, 0:256], add)
    nc.vector.tensor_tensor(gp[0:100, 256:512], gp[0:100, 256:512],
                            L.prew_cur[0:100, s_pre, 256:512], add)
    up = L.tmp_pool.tile([128, 256], fp32, tag="up", name="up")
    nc.scalar.activation(up[0:100, :], gp[0:100, 0:256],
                         mybir.ActivationFunctionType.Sigmoid, scale=-1.0)
    d = L.tmp_pool.tile([128, 256], fp32, tag="d", name="d")
    nc.vector.scalar_tensor_tensor(
        d[0:100, :], gp[0:100, 256:512], 0.0, L.prev_h[0:100, :],
        mybir.AluOpType.max, mybir.AluOpType.subtract)
    nc.vector.tensor_tensor(d[0:100, :], d[0:100, :], up[0:100, :],
                            mybir.AluOpType.mult)
    nc.vector.tensor_tensor(hn[0:100, :], L.prev_h[0:100, :],
                            d[0:100, :], add)

    # ---- rebuild h^T: identity matmuls on 4 concurrent row-groups
    tp = L.tps_pool.tile([128, 32], fp32, tag="tp", name="tp")
    for k in range(8):
        j, kk = k // 2, k % 2
        nc.tensor.matmul(
            tp[:, 4 * k: 4 * k + 4],
            hn[32 * j: 32 * j + 4, 128 * kk: 128 * kk + 128],
            L.idt_t[32 * j: 32 * j + 4, 0:4],
            tile_position=(32 * j, 0),
            skip_group_check=True,
        )
    if layer == 0:
        nc.vector.tensor_copy(
            L.o0t_cur[:, :, 4 * sw: 4 * sw + 4],
            tp.rearrange("p (k b) -> p k b", b=4))
        ht_cur = L.o0t_cur[:, :, 4 * sw: 4 * sw + 4]
    else:
        ht_cur = L.ht_pool.tile([128, 32], bf16, tag="ht", name="ht")
        nc.vector.tensor_copy(ht_cur[:, :], tp[:, :])
    if layer == 0:
        pass
        if sw == P1_WIN - 1:
            p1sb = L.psb_pool.tile([128, 2048], fp32, tag="psb", name="psb")
            for half in range(2):
                pp = L.pps_pool.tile([128, 1024], fp32, tag="pps", name="pps")
                for n in (2 * half, 2 * half + 1):
                    for k in range(8):
                        nc.tensor.matmul(
                            pp[:, 512 * (n - 2 * half): 512 * (n - 2 * half) + 512],
                            L.o0t_cur[:, k, :],
                            L.wih1p_t[k][:, 512 * n: 512 * n + 512],
                            start=(k == 0), stop=(k == 7),
                        )
                nc.vector.tensor_tensor(
                    p1sb[:, 1024 * half: 1024 * half + 1024], pp[:, :],
                    L.b1f_t[:, 1024 * half: 1024 * half + 1024], add)
            nc.sync.dma_start(L.pre1_d[ts // P1_WIN, :, :], p1sb[:, :])
    else:
        if s_st == OUT_WIN - 1:
            w0 = ts - (OUT_WIN - 1)
            for j in range(4):
                nc.sync.dma_start(
                    L.out_dram[:, w0: w0 + OUT_WIN, 256 * j: 256 * j + 256],
                    L.stage_cur[32 * j: 32 * j + 4, :, :],
                )

    L.prev_h = hn
    L.ht_src = ht_cur


def _build():
    fp32 = mybir.dt.float32
    bf16 = mybir.dt.bfloat16
    nc = bacc.Bacc("TRN2", target_bir_lowering=False, debug=False,
                   num_devices=NCORES)

    xt = nc.dram_tensor("xt", [DX, BL * T], bf16, kind="ExternalInput")
    w0p = nc.dram_tensor("w0p", [H, G], bf16, kind="ExternalInput")
    w1p = nc.dram_tensor("w1p", [H, G], bf16, kind="ExternalInput")
    wih0p = nc.dram_tensor("wih0p", [DX, G], bf16, kind="ExternalInput")
    wih1p = nc.dram_tensor("wih1p", [H, G], bf16, kind="ExternalInput")
    b0f = nc.dram_tensor("b0f", [128, G], fp32, kind="ExternalInput")
    b1f = nc.dram_tensor("b1f", [128, G], fp32, kind="ExternalInput")
    h0t = nc.dram_tensor("h0t", [128, 32], bf16, kind="ExternalInput")
    h1t = nc.dram_tensor("h1t", [128, 32], bf16, kind="ExternalInput")
    h0s = nc.dram_tensor("h0s", [128, 256], fp32, kind="ExternalInput")
    h1s = nc.dram_tensor("h1s", [128, 256], fp32, kind="ExternalInput")
    idt = nc.dram_tensor("idt", [128, 4], fp32, kind="ExternalInput")
    out = nc.dram_tensor("out", [BL, T, H], fp32, kind="ExternalOutput")

    pre0_d = nc.dram_tensor("pre0_d", [BL, T, G], fp32, kind="Internal")
    pre1_d = nc.dram_tensor("pre1_d", [T // P1_WIN, 128, G], fp32,
                            kind="Internal")

    with TileContext(nc) as tc:
        with tc.tile_pool(name="wconst", bufs=1) as wconst:
            w0_t = [wconst.tile([128, G], bf16, tag=f"w0_{k}", name=f"w0_{k}")
                    for k in range(8)]
            w1_t = [wconst.tile([128, G], bf16, tag=f"w1_{k}", name=f"w1_{k}")
                    for k in range(8)]
            wih1p_t = [wconst.tile([128, G], bf16, tag=f"wih1_{k}",
                                   name=f"wih1_{k}") for k in range(8)]
            for k in range(8):
                nc.sync.dma_start(w0_t[k][:, :], w0p[128 * k: 128 * k + 128, :])
                nc.sync.dma_start(w1_t[k][:, :], w1p[128 * k: 128 * k + 128, :])
                nc.sync.dma_start(wih1p_t[k][:, :],
                                  wih1p[128 * k: 128 * k + 128, :])
            b1f_t = wconst.tile([128, G], fp32, tag="b1f", name="b1f")
            nc.sync.dma_start(b1f_t[:, :], b1f[:, :])
            h0t_t = wconst.tile([128, 32], bf16, tag="h0t", name="h0t")
            h1t_t = wconst.tile([128, 32], bf16, tag="h1t", name="h1t")
            h0s_t = wconst.tile([128, 256], fp32, tag="h0s", name="h0s")
            h1s_t = wconst.tile([128, 256], fp32, tag="h1s", name="h1s")
            idt_t = wconst.tile([128, 4], fp32, tag="idt", name="idt")
            for dst, src in ((h0t_t, h0t), (h1t_t, h1t), (h0s_t, h0s),
                             (h1s_t, h1s), (idt_t, idt)):
                nc.sync.dma_start(dst[:, :], src[:, :])

            # ---- P1: layer-0 input GEMM, shuffled out to pre0_d
            with (
                tc.tile_pool(name="p1x", bufs=1) as p1x,
                tc.tile_pool(name="p1ps", bufs=2, space="PSUM") as p1ps,
                tc.tile_pool(name="p1o", bufs=2) as p1o,
            ):
                b0f_t = p1x.tile([128, G], fp32, tag="b0f", name="b0f")
                nc.sync.dma_start(b0f_t[:, :], b0f[:, :])
                xt_t = [p1x.tile([128, BL * T], bf16, tag=f"xt{k}",
                                 name=f"xtt{k}") for k in range(4)]
                wih0_t = [p1x.tile([128, G], bf16, tag=f"wih0_{k}",
                                   name=f"wih0_{k}") for k in range(4)]
                for k in range(4):
                    nc.sync.dma_start(xt_t[k][:, :],
                                      xt[128 * k: 128 * k + 128, :])
                    nc.sync.dma_start(wih0_t[k][:, :],
                                      wih0p[128 * k: 128 * k + 128, :])
                for m in range(BL * T // 128):
                    pp = p1ps.tile([128, G], fp32, tag="pp", name="pp")
                    for n in range(4):
                        for k in range(4):
                            nc.tensor.matmul(
                                pp[:, 512 * n: 512 * n + 512],
                                xt_t[k][:, 128 * m: 128 * m + 128],
                                wih0_t[k][:, 512 * n: 512 * n + 512],
                                start=(k == 0), stop=(k == 3),
                            )
                    po = p1o.tile([128, G], fp32, tag="po", name="po")
                    nc.vector.tensor_tensor(po[:, :], pp[:, :], b0f_t[:, :],
                                            mybir.AluOpType.add)
                    bb = m // (T // 128)
                    t0 = 128 * (m % (T // 128))
                    nc.sync.dma_start(pre0_d[bb, t0: t0 + 128, :], po[:, :])

            tc.strict_bb_all_engine_barrier()
            with contextlib.ExitStack() as stack:
                L0 = _make_layer(nc, tc, stack, 0, w0_t, pre0_d, h0t_t,
                                 h0s_t, idt_t, wih1p_t=wih1p_t, b1f_t=b1f_t,
                                 pre1_d=pre1_d)
                L1 = _make_layer(nc, tc, stack, 1, w1_t, pre1_d, h1t_t,
                                 h1s_t, idt_t, out_dram=out)
                for tt in range(T + LAG):
                    if tt < T:
                        _emit_step(nc, tc, L0, tt)
                    if tt >= LAG:
                        _emit_step(nc, tc, L1, tt - LAG)

    nc.compile()
    return nc


def _prep_core(inputs, c, shared):
    x = inputs["x"][BL * c: BL * c + BL, :T]          # [4, T, DX]
    xt = np.ascontiguousarray(
        x.transpose(2, 0, 1).reshape(DX, BL * T)).astype(BF16)

    def hscat(hv):                                    # [4, H] -> [128, 256]
        o = np.zeros((128, 256), F32)
        for j in range(4):
            o[32 * j: 32 * j + 4, :] = hv[:, 256 * j: 256 * j + 256]
        return o

    def htr(hv):                                      # [4, H] -> [128, 32]
        o = np.zeros((128, 32), F32)
        for k in range(8):
            o[:, 4 * k: 4 * k + 4] = hv[:, 128 * k: 128 * k + 128].T
        return o

    h0 = inputs["hx"][0, BL * c: BL * c + BL]
    h1 = inputs["hx"][1, BL * c: BL * c + BL]
    return {
        "xt": xt,
        "h0t": htr(h0).astype(BF16), "h1t": htr(h1).astype(BF16),
        "h0s": hscat(h0), "h1s": hscat(h1),
        **shared,
    }


def get_nc():
    nc = _CACHE.get("nc")
    if nc is None:
        nc = _build()
        _CACHE["nc"] = nc
    return nc


def make_in_maps(inputs):
    inputs = {k: np.asarray(v) for k, v in inputs.items()}
    perm = _CACHE.setdefault("perm", _perm())
    idt = np.zeros((128, 4), F32)
    for j in range(4):
        for b in range(4):
            idt[32 * j + b, b] = 1.0
    shared = {
        "w0p": np.ascontiguousarray(inputs["w_hh_l0"][perm].T).astype(BF16),
        "w1p": np.ascontiguousarray(inputs["w_hh_l1"][perm].T).astype(BF16),
        "wih0p": np.ascontiguousarray(inputs["w_ih_l0"][perm].T).astype(BF16),
        "wih1p": np.ascontiguousarray(inputs["w_ih_l1"][perm].T).astype(BF16),
        "b0f": np.broadcast_to(
            (inputs["b_ih_l0"] + inputs["b_hh_l0"])[perm],
            (128, G)).astype(F32).copy(),
        "b1f": np.broadcast_to(
            (inputs["b_ih_l1"] + inputs["b_hh_l1"])[perm],
            (128, G)).astype(F32).copy(),
        "idt": idt,
    }
    return [_prep_core(inputs, c, shared) for c in range(NCORES)]


def kernel(**inputs) -> np.ndarray:
    nc = get_nc()
    in_maps = make_in_maps(inputs)
    try:
        res = run_bass_kernel_spmd(nc, in_maps, core_ids=list(range(NCORES)))
    except Exception:
        # a previously wedged device often recovers on the next attempt
        import time
        time.sleep(2.0)
        res = run_bass_kernel_spmd(nc, in_maps, core_ids=list(range(NCORES)))
    out = np.concatenate([res.results[c]["out"] for c in range(NCORES)],
                         axis=0)
    return np.asarray(out, np.float32)

